# revision 20
# baseline (speedup 1.0000x reference)
"""Trainium2 Bass kernel for nn_DecoderRNN (GRU decoder, 140 sequential steps).

Strategy (data-parallel, per sharding hint):
  - B=512 sharded 8 ways -> 64 batch rows per core; weights replicated.
  - Feature-major on-chip layout: a [F, B] tensor is F/128 chunks of
    [128 partitions, 64 batch] side by side in the free dim.
  - Weight fusion: the fed-back x = h @ out_W.T + out_b is linear in h, so
    it is folded into the next step's embedding (M1 = out_W.T @ emb_W.T)
    and into the decoder output projection (Wyd = out_W.T @ reg_W.T).
  - Gate biases land in PSUM via ONE indicator matmul per bank per step:
    lhsT rows 0..3 hold the 4 quarter-bias vectors, rhs is a constant
    indicator pattern ind[k, j] = (j//64 == k), so out[p, j] = bias_{j//64}[p].
  - relu runs on DVE (tensor_scalar_max) to unload the ACT engine; ACT keeps
    sigmoid_r, tanh, sigmoid_z and the y PSUM->SBUF copy.
  - PE phase order A,Br,Cr,D,E,Bz,Cz puts the z gate last (its post-closure
    chain depth sigmoid_z->t6->h' is the shortest), y matmuls execute during
    the sigmoid_z window, and the NEXT step's 6 bias matmuls are emitted
    after the DVE tail so the PE has work while h' is being produced.
"""

import numpy as np
import ml_dtypes

B, T_ENC, E, H, O, PRED_LEN = 512, 140, 256, 512, 64, 140
NCORES = 8
BC = B // NCORES           # 64 batch rows per core
T_ALL = T_ENC + PRED_LEN   # 280

bf16 = ml_dtypes.bfloat16


def _pack_tiles(wT, n_k, n_m):
    """Pack a [K, M] (pre-transposed) weight into [128, n_m*n_k*128] bf16:
    tile (mi, k) at cols (mi*n_k + k)*128."""
    K, M = wT.shape
    assert K == n_k * 128 and M == n_m * 128
    t = wT.reshape(n_k, 128, n_m, 128).transpose(2, 0, 1, 3)  # [mc, kc, 128, 128]
    t = t.transpose(2, 0, 1, 3).reshape(128, -1)
    return np.ascontiguousarray(t.astype(bf16))


def _feat_major(x, n_chunks):
    """[B, F] -> [128, n_chunks*B] feature-major chunk layout."""
    b, f = x.shape
    assert f == n_chunks * 128
    t = x.reshape(b, n_chunks, 128).transpose(2, 1, 0).reshape(128, n_chunks * b)
    return np.ascontiguousarray(t)


def build_program(nsteps=PRED_LEN, t_enc=T_ENC, lowering=True):
    """Build the Bass program (per-core SPMD). Returns nc."""
    import concourse.bass as bass
    import concourse.tile as tile
    from concourse import bacc, mybir

    AF = mybir.ActivationFunctionType
    OP = mybir.AluOpType
    f32 = mybir.dt.float32
    bf = mybir.dt.bfloat16

    if lowering:
        nc = bacc.Bacc("TRN2", target_bir_lowering=True, debug=False)
    else:
        nc = bass.Bass("TRN2", target_bir_lowering=False, debug=False)

    # ---- DRAM I/O ----
    encT_d = nc.dram_tensor("encT", [128, t_enc * 128], bf, kind="ExternalInput").ap()
    h0_d = nc.dram_tensor("h0T", [128, 4 * BC], bf, kind="ExternalInput").ap()
    m1_d = nc.dram_tensor("m1T", [128, 16 * 128], bf, kind="ExternalInput").ap()
    wih_d = nc.dram_tensor("wihT", [128, 48 * 128], bf, kind="ExternalInput").ap()
    whh_d = nc.dram_tensor("whhT", [128, 48 * 128], bf, kind="ExternalInput").ap()
    emb_d = nc.dram_tensor("embT", [128, 8 * 128], bf, kind="ExternalInput").ap()
    wyd_d = nc.dram_tensor("wydT", [128, 4 * O], bf, kind="ExternalInput").ap()
    regw_d = nc.dram_tensor("regwT", [128, 2 * O], bf, kind="ExternalInput").ap()
    # bias tiles [128,128]: tile j, partition-row k<4 holds the k-th quarter
    # of that bias vector.  Order: E(c_e) E0(emb_b) R Z HN IN
    bias_d = nc.dram_tensor("biasT", [128, 6 * 128], bf, kind="ExternalInput").ap()
    # indicator rhs: cols 0:256 ind[k, j] = 1 iff j//64 == k (k<4);
    # cols 256:384 ind[k, j] = 1 iff k == 2 + j//64 (for the e_hi bank)
    ind_d = nc.dram_tensor("indT", [128, 384], bf, kind="ExternalInput").ap()
    # y-bias rhs tiles [128, 2*O]: row 0 of block 0 = reg_b, block 1 = c_yd
    ybias_d = nc.dram_tensor("ybT", [128, 2 * O], bf, kind="ExternalInput").ap()
    y_d = nc.dram_tensor("y", [BC, t_enc + nsteps, O], f32, kind="ExternalOutput").ap()

    with tile.TileContext(nc) as tc:
        import contextlib
        with contextlib.ExitStack() as ctx:
            consts = ctx.enter_context(tc.tile_pool(name="consts", bufs=1))
            temps = ctx.enter_context(tc.tile_pool(name="temps", bufs=2))
            ytmp = ctx.enter_context(tc.tile_pool(name="ytmp", bufs=3))
            psum = ctx.enter_context(tc.tile_pool(name="psum", bufs=1, space="PSUM"))

            # ---- ACT table warmup (pin the table load to dependency-light
            # dummy ops) ----
            wt = consts.tile([128, 10], f32, tag="wtbl", name="wtbl")
            nc.vector.memset(wt[:, 0:5], 0.0)
            nc.scalar.activation(wt[:, 5:6], wt[:, 0:1], AF.Relu)
            nc.scalar.activation(wt[:, 6:7], wt[:, 1:2], AF.Sigmoid)
            nc.scalar.activation(wt[:, 7:8], wt[:, 2:3], AF.Tanh)
            nc.scalar.activation(wt[:, 8:9], wt[:, 3:4], AF.Identity)

            # ---- load constants into SBUF ----
            m1_sb = consts.tile([128, 16 * 128], bf, tag="m1")
            wih_sb = consts.tile([128, 48 * 128], bf, tag="wih")
            whh_sb = consts.tile([128, 48 * 128], bf, tag="whh")
            emb_sb = consts.tile([128, 8 * 128], bf, tag="emb")
            wyd_sb = consts.tile([128, 4 * O], bf, tag="wyd")
            regw_sb = consts.tile([128, 2 * O], bf, tag="regw")
            bias_sb = consts.tile([128, 6 * 128], bf, tag="biasT")
            ind_sb = consts.tile([128, 384], bf, tag="indT")
            ybias_sb = consts.tile([128, 2 * O], bf, tag="ybT")
            encT_sb = consts.tile([128, t_enc * 128], bf, tag="encT")

            nc.sync.dma_start(out=emb_sb, in_=emb_d)
            nc.sync.dma_start(out=bias_sb, in_=bias_d)
            nc.sync.dma_start(out=ind_sb, in_=ind_d)
            nc.sync.dma_start(out=m1_sb, in_=m1_d)
            nc.sync.dma_start(out=whh_sb, in_=whh_d)
            nc.sync.dma_start(out=wih_sb, in_=wih_d)
            nc.sync.dma_start(out=wyd_sb, in_=wyd_d)
            nc.sync.dma_start(out=regw_sb, in_=regw_d)
            nc.sync.dma_start(out=ybias_sb, in_=ybias_d)
            # x0 block (last encoder token) first so step 0 can start early
            lastblk = slice((t_enc - 1) * 128, t_enc * 128)
            nc.sync.dma_start(out=encT_sb[:, lastblk], in_=encT_d[:, lastblk])
            nsplit = 4
            per = (t_enc - 1) // nsplit + 1
            for i in range(nsplit):
                lo, hi = i * per, min((i + 1) * per, t_enc - 1)
                if lo >= hi:
                    continue
                nc.sync.dma_start(out=encT_sb[:, lo * 128:hi * 128],
                                  in_=encT_d[:, lo * 128:hi * 128])

            # ---- persistent state: h + a constant ones chunk at [4BC:5BC]
            # (lhsT for the y-bias matmul) ----
            h_sb = consts.tile([128, 5 * BC], bf, tag="h", name="h")
            nc.sync.dma_start(out=h_sb[:, 0:4 * BC], in_=h0_d)
            nc.vector.memset(h_sb[:, 4 * BC:5 * BC], 1.0)
            h_ones = h_sb[:, 4 * BC:5 * BC]

            # ---- persistent PSUM regions, one bank each.  The e pre-act is
            # split across TWO banks (quarters 0,1 / 2,3) so relu_lo can
            # start as soon as the first half of A closes, shortening the
            # A -> relu -> Cr chain prefix.  y gets a single bank to pay
            # for it (its copy/bias WAR coupling has ample slack). ----
            el_ps = psum.tile([128, 512], f32, tag="elps")     # e quarters 0,1
            eh_ps = psum.tile([128, 512], f32, tag="ehps")     # e quarters 2,3
            r_ps = psum.tile([128, 512], f32, tag="rps")       # a_r
            hn_ps = psum.tile([128, 512], f32, tag="hnps")     # hn
            in_ps = psum.tile([128, 512], f32, tag="inps")     # inn
            z_pp = [psum.tile([128, 512], f32, tag=f"zps{s}", name=f"zps{s}")
                    for s in range(2)]
            y_ps = psum.tile([BC, 512], f32, tag="yps")

            def wtile(sb, mi, k, n_k):
                j = (mi * n_k + k) * 128
                return sb[:, j:j + 128]

            # bias tile indices in bias_sb
            BE, BE0, BR, BZ, BHN, BIN = 0, 1, 2, 3, 4, 5

            def btile(idx):
                return bias_sb[:, idx * 128:(idx + 1) * 128]

            cs = lambda m: slice(m * BC, (m + 1) * BC)

            def emit_biases(t):
                """Arm all PSUM banks for step t with their bias values.
                Emitted in step t-1's tail: none has an outstanding
                dependency there, so they fill the PE's h'-wait window."""
                eb = BE0 if t == 0 else BE
                nc.tensor.matmul(el_ps[:, 0:128], btile(eb), ind_sb[:, 0:128],
                                 start=True, stop=False)
                nc.tensor.matmul(eh_ps[:, 0:128], btile(eb),
                                 ind_sb[:, 256:384], start=True, stop=False)
                nc.tensor.matmul(r_ps[:, 0:256], btile(BR), ind_sb[:, 0:256],
                                 start=True, stop=False)
                nc.tensor.matmul(hn_ps[:, 0:256], btile(BHN), ind_sb[:, 0:256],
                                 start=True, stop=False)
                nc.tensor.matmul(in_ps[:, 0:256], btile(BIN), ind_sb[:, 0:256],
                                 start=True, stop=False)
                nc.tensor.matmul(z_pp[t % 2][:, 0:256], btile(BZ),
                                 ind_sb[:, 0:256], start=True, stop=False)

            def gate_mms(dst, w_sb, m_base, src, n_k, stop_last,
                         k_major=False):
                """One gate region: 4 output quarters x n_k contraction
                chunks, continuing the bank group armed by its bias mm.
                k_major orders the k=0,1 matmuls first so a relu_lo-gated
                group can start before relu_hi lands."""
                order = [(m, k) for k in range(n_k) for m in range(4)] \
                    if k_major else [(m, k) for m in range(4)
                                     for k in range(n_k)]
                for i, (m, k) in enumerate(order):
                    nc.tensor.matmul(dst[:, cs(m)],
                                     wtile(w_sb, m_base + m, k, n_k),
                                     src[:, k * BC:(k + 1) * BC],
                                     start=False,
                                     stop=(stop_last and i == len(order) - 1))

            def emit_e_mms(t):
                """A-group, split across the two e banks: quarters 0,1 into
                el_ps first (closing it early for relu_lo), then 2,3."""
                w_sb, n_k = (emb_sb, 2) if t == 0 else (m1_sb, 4)
                src = encT_sb[:, lastblk] if t == 0 else h_sb
                for half, dst in ((0, el_ps), (1, eh_ps)):
                    for m in range(2):
                        for k in range(n_k):
                            nc.tensor.matmul(
                                dst[:, cs(m)],
                                wtile(w_sb, half * 2 + m, k, n_k),
                                src[:, k * BC:(k + 1) * BC],
                                start=False,
                                stop=(m == 1 and k == n_k - 1))

            emit_biases(0)
            nc.tensor.matmul(y_ps[:, 0:128], h_ones, ybias_sb,
                             start=True, stop=False)

            for t in range(nsteps):
                e_t = temps.tile([128, 256], bf, tag="e")
                r_t = temps.tile([128, 256], bf, tag="r")
                z_t = temps.tile([128, 256], bf, tag="z")
                n_t = temps.tile([128, 256], bf, tag="n")
                t3 = temps.tile([128, 256], bf, tag="t3")
                d_t = temps.tile([128, 256], bf, tag="d")
                t6 = temps.tile([128, 256], bf, tag="t6")

                z_ps = z_pp[t % 2]

                # A: e pre-activation (h-gated; step 0 uses emb @ x0),
                # half el closes first
                emit_e_mms(t)
                # Br: whh r-half (h-gated, group stays open)
                gate_mms(r_ps, whh_sb, 0, h_sb, 4, False)
                # relu on DVE, by e-bank half (unloads ACT; lets Cr's k=0,1
                # matmuls start before the hi half lands)
                nc.vector.tensor_scalar_max(e_t[:, 0:128], el_ps[:, 0:128], 0.0)
                nc.vector.tensor_scalar_max(e_t[:, 128:256], eh_ps[:, 0:128], 0.0)
                # Cr: wih r-half closes the r bank (relu-gated, k-major)
                gate_mms(r_ps, wih_sb, 0, e_t, 4, True, k_major=True)
                # sigma_r (ACT)
                nc.scalar.activation(r_t, r_ps[:, 0:256], AF.Sigmoid)
                # D: hn region (h-gated)
                gate_mms(hn_ps, whh_sb, 8, h_sb, 4, True)
                # E: inn region (relu-gated)
                gate_mms(in_ps, wih_sb, 8, e_t, 4, True)
                # t3 = hn * r ; t4 = t3 + inn  (DVE).  t4 lands in the hn
                # bank's free upper half so tanh reads PSUM (faster ScE src
                # than SBUF); the hn bank is quiet there (t3 already read
                # its lower half, the next writer is hn-bias(t+1) whose WAR
                # on tanh orders it correctly).
                nc.vector.tensor_tensor(t3, hn_ps[:, 0:256], r_t, OP.mult)
                nc.vector.tensor_tensor(hn_ps[:, 256:512], t3,
                                        in_ps[:, 0:256], OP.add)
                # Bz / Cz: z gate last (shortest post-closure chain depth)
                gate_mms(z_ps, whh_sb, 4, h_sb, 4, False)
                gate_mms(z_ps, wih_sb, 4, e_t, 4, True)
                # n = tanh(t4); sigma_z after it in the ACT queue
                nc.scalar.activation(n_t, hn_ps[:, 256:512], AF.Tanh)
                nc.scalar.activation(z_t, z_ps[:, 0:256], AF.Sigmoid)
                # y matmuls (read h(t) = current h_sb, so they must be
                # emitted before the DVE tail overwrites it; they execute
                # during the sigma_z window)
                if t > 0:
                    for k in range(4):
                        nc.tensor.matmul(y_ps[:, 64:128],
                                         h_sb[:, k * BC:(k + 1) * BC],
                                         wyd_sb[:, k * O:(k + 1) * O],
                                         start=False,
                                         stop=(k == 3 and t >= t_enc))
                if t < t_enc:
                    for k in range(2):
                        nc.tensor.matmul(
                            y_ps[:, 0:64],
                            encT_sb[:, t * 128 + k * BC: t * 128 + (k + 1) * BC],
                            regw_sb[:, k * O:(k + 1) * O],
                            start=False, stop=(k == 1))
                # DVE tail: d = h - n; t6 = z*d; h' = n + t6 (in-place)
                nc.vector.tensor_tensor(d_t, h_sb[:, 0:256], n_t, OP.subtract)
                nc.vector.tensor_tensor(t6, z_t, d_t, OP.mult)
                nc.vector.tensor_tensor(h_sb[:, 0:256], n_t, t6, OP.add)
                # next step's bias matmuls: dependency-free here, they fill
                # the PE's wait-for-h' window
                if t + 1 < nsteps:
                    emit_biases(t + 1)

                # ---------------- y staging + DMA ----------------
                if t > 0 and t < t_enc:
                    y_all = ytmp.tile([BC, 128], f32, tag="yall")
                    nc.scalar.activation(y_all, y_ps[:, 0:128], AF.Identity)
                    nc.sync.dma_start(out=y_d[:, t, :], in_=y_all[:, 0:64])
                    nc.sync.dma_start(out=y_d[:, t_enc + t - 1, :],
                                      in_=y_all[:, 64:128])
                elif t < t_enc:  # t == 0: encoder token only
                    y_enc = ytmp.tile([BC, O], f32, tag="yenc")
                    nc.scalar.activation(y_enc, y_ps[:, 0:64], AF.Identity)
                    nc.sync.dma_start(out=y_d[:, t, :], in_=y_enc)
                else:            # t >= t_enc: decoder token only
                    y_dec = ytmp.tile([BC, O], f32, tag="ydec")
                    nc.scalar.activation(y_dec, y_ps[:, 64:128], AF.Identity)
                    nc.sync.dma_start(out=y_d[:, t_enc + t - 1, :], in_=y_dec)
                # y-bias for step t+1 (single y bank: must be emitted AFTER
                # this step's ycopy so its WAR pins it behind the copy --
                # the scheduler would otherwise hoist the start=True matmul
                # over this step's y accumulation and wipe it)
                if t + 1 < nsteps:
                    nc.tensor.matmul(y_ps[:, 0:128], h_ones, ybias_sb,
                                     start=True, stop=False)

            # final decoder token from h(nsteps)
            nc.tensor.matmul(y_ps[:, 64:128], h_ones, ybias_sb[:, O:2 * O],
                             start=True, stop=False)
            for k in range(4):
                nc.tensor.matmul(y_ps[:, 64:128],
                                 h_sb[:, k * BC:(k + 1) * BC],
                                 wyd_sb[:, k * O:(k + 1) * O],
                                 start=False, stop=(k == 3))
            y_fin = ytmp.tile([BC, O], f32, tag="ydec")
            nc.scalar.activation(y_fin, y_ps[:, 64:128], AF.Identity)
            nc.sync.dma_start(out=y_d[:, t_enc + nsteps - 1, :], in_=y_fin)

            # leftover encoder tokens if nsteps < t_enc (smoke tests only)
            for t in range(nsteps, t_enc):
                nc.tensor.matmul(y_ps[:, 0:64], h_ones, ybias_sb[:, 0:O],
                                 start=True, stop=False)
                for k in range(2):
                    nc.tensor.matmul(
                        y_ps[:, 0:64],
                        encT_sb[:, t * 128 + k * BC: t * 128 + (k + 1) * BC],
                        regw_sb[:, k * O:(k + 1) * O],
                        start=False, stop=(k == 1))
                y_enc = ytmp.tile([BC, O], f32, tag="yenc")
                nc.scalar.activation(y_enc, y_ps[:, 0:64], AF.Identity)
                nc.sync.dma_start(out=y_d[:, t, :], in_=y_enc)

    if lowering:
        nc.finalize()
    return nc


def prep_inputs(encoder_outputs, encoder_hidden, emb_W, emb_b, w_ih, w_hh,
                b_ih, b_hh, out_W, out_b, reg_W, reg_b, nsteps=PRED_LEN,
                t_enc=T_ENC):
    """Host-side packing. Returns per-core input dicts."""
    f32 = np.float32
    f64 = np.float64
    emb_W, emb_b, w_ih, w_hh, b_ih, b_hh, out_W, out_b, reg_W, reg_b = (
        np.asarray(a, f32) for a in
        (emb_W, emb_b, w_ih, w_hh, b_ih, b_hh, out_W, out_b, reg_W, reg_b))

    # fused weights (x = h@out_W.T + out_b is linear in h)
    M1 = (out_W.T.astype(f64) @ emb_W.T.astype(f64)).astype(f32)     # [H, H]
    c_e = (emb_b + out_b @ emb_W.T).astype(f32)                      # [H]
    Wyd = (out_W.T.astype(f64) @ reg_W.T.astype(f64)).astype(f32)    # [H, O]
    c_yd = (out_b @ reg_W.T + reg_b).astype(f32)                     # [O]

    # bias tiles [128, 6*128]: tile j, partition-row k<4 = quarter k of the
    # bias vector (order E, E0, R, Z, HN, IN)
    bias_rows = np.stack([
        c_e.reshape(4, 128),
        emb_b.reshape(4, 128),
        (b_ih[:H] + b_hh[:H]).reshape(4, 128),
        (b_ih[H:2 * H] + b_hh[H:2 * H]).reshape(4, 128),
        b_hh[2 * H:].reshape(4, 128),
        b_ih[2 * H:].reshape(4, 128),
    ])                                                   # [6, 4, 128]
    bias_pack = np.zeros((128, 6 * 128), f32)
    for j in range(6):
        for k in range(4):
            bias_pack[k, j * 128:(j + 1) * 128] = bias_rows[j, k]

    # indicator rhs: cols 0:256 ind[k, j] = 1 iff j//64 == k; cols 256:384
    # select quarters 2,3 for the e_hi bank
    ind_pack = np.zeros((128, 384), f32)
    for k in range(4):
        ind_pack[k, k * 64:(k + 1) * 64] = 1.0
    for k in range(2):
        ind_pack[2 + k, 256 + k * 64:256 + (k + 1) * 64] = 1.0

    yb_pack = np.zeros((128, 2 * O), f32)
    yb_pack[0, 0:O] = reg_b
    yb_pack[0, O:2 * O] = c_yd

    shared = {
        "m1T": _pack_tiles(M1, 4, 4),
        "wihT": _pack_tiles(w_ih.T, 4, 12),
        "whhT": _pack_tiles(w_hh.T, 4, 12),
        "embT": _pack_tiles(emb_W.T, 2, 4),
        "wydT": np.ascontiguousarray(
            Wyd.reshape(4, 128, O).transpose(1, 0, 2).reshape(128, 4 * O)
            .astype(bf16)),
        "regwT": np.ascontiguousarray(
            reg_W.T.reshape(2, 128, O).transpose(1, 0, 2).reshape(128, 2 * O)
            .astype(bf16)),
        "biasT": np.ascontiguousarray(bias_pack.astype(bf16)),
        "indT": np.ascontiguousarray(ind_pack.astype(bf16)),
        "ybT": np.ascontiguousarray(yb_pack.astype(bf16)),
    }

    enc = np.asarray(encoder_outputs, f32)[:, :t_enc, :]
    h0 = np.asarray(encoder_hidden, f32)[0]
    in_maps = []
    for i in range(NCORES):
        sl = slice(i * BC, (i + 1) * BC)
        enc_i = enc[sl].astype(bf16)              # [BC, t_enc, E]
        encT = (enc_i.reshape(BC, t_enc, 2, 128).transpose(3, 1, 2, 0)
                .reshape(128, t_enc * 128))
        m = dict(shared)
        m["encT"] = np.ascontiguousarray(encT)
        m["h0T"] = _feat_major(h0[sl], 4).astype(bf16)
        in_maps.append(m)
    return in_maps


def kernel(encoder_outputs, encoder_hidden, emb_W, emb_b, w_ih, w_hh,
           b_ih, b_hh, out_W, out_b, reg_W, reg_b):
    from concourse.bass_utils import run_bass_kernel_spmd

    nc = build_program()
    in_maps = prep_inputs(encoder_outputs, encoder_hidden, emb_W, emb_b,
                          w_ih, w_hh, b_ih, b_hh, out_W, out_b, reg_W, reg_b)
    res = run_bass_kernel_spmd(nc, in_maps, core_ids=list(range(NCORES)))
    out = np.empty((B, T_ALL, O), np.float32)
    for i in range(NCORES):
        out[i * BC:(i + 1) * BC] = res.results[i]["y"]
    return out


# revision 27
# speedup vs baseline: 1.0169x; 1.0169x over previous
"""Trainium2 Bass kernel for nn_DecoderRNN (GRU decoder, 140 sequential steps).

Strategy (data-parallel, per sharding hint):
  - B=512 sharded 8 ways -> 64 batch rows per core; weights replicated.
  - Feature-major on-chip layout: a [F, B] tensor is F/128 chunks of
    [128 partitions, 64 batch] side by side in the free dim.
  - Weight fusion: the fed-back x = h @ out_W.T + out_b is linear in h, so
    it is folded into the next step's embedding (M1 = out_W.T @ emb_W.T)
    and into the decoder output projection (Wyd = out_W.T @ reg_W.T).
  - Gate biases land in PSUM via ONE indicator matmul per bank per step:
    lhsT rows 0..3 hold the 4 quarter-bias vectors, rhs is a constant
    indicator pattern ind[k, j] = (j//64 == k), so out[p, j] = bias_{j//64}[p].
  - relu runs on DVE (tensor_scalar_max) to unload the ACT engine; ACT keeps
    sigmoid_r, tanh, sigmoid_z and the y PSUM->SBUF copy.
  - PE phase order A,Br,Cr,D,E,Bz,Cz puts the z gate last (its post-closure
    chain depth sigmoid_z->t6->h' is the shortest), y matmuls execute during
    the sigmoid_z window, and the NEXT step's 6 bias matmuls are emitted
    after the DVE tail so the PE has work while h' is being produced.
"""

import numpy as np
import ml_dtypes

B, T_ENC, E, H, O, PRED_LEN = 512, 140, 256, 512, 64, 140
NCORES = 8
BC = B // NCORES           # 64 batch rows per core
T_ALL = T_ENC + PRED_LEN   # 280

bf16 = ml_dtypes.bfloat16


def _pack_tiles(wT, n_k, n_m):
    """Pack a [K, M] (pre-transposed) weight into [128, n_m*n_k*128] bf16:
    tile (mi, k) at cols (mi*n_k + k)*128."""
    K, M = wT.shape
    assert K == n_k * 128 and M == n_m * 128
    t = wT.reshape(n_k, 128, n_m, 128).transpose(2, 0, 1, 3)  # [mc, kc, 128, 128]
    t = t.transpose(2, 0, 1, 3).reshape(128, -1)
    return np.ascontiguousarray(t.astype(bf16))


def _feat_major(x, n_chunks):
    """[B, F] -> [128, n_chunks*B] feature-major chunk layout."""
    b, f = x.shape
    assert f == n_chunks * 128
    t = x.reshape(b, n_chunks, 128).transpose(2, 1, 0).reshape(128, n_chunks * b)
    return np.ascontiguousarray(t)


def build_program(nsteps=PRED_LEN, t_enc=T_ENC, lowering=True):
    """Build the Bass program (per-core SPMD). Returns nc."""
    import concourse.bass as bass
    import concourse.tile as tile
    from concourse import bacc, mybir

    AF = mybir.ActivationFunctionType
    OP = mybir.AluOpType
    f32 = mybir.dt.float32
    bf = mybir.dt.bfloat16

    if lowering:
        nc = bacc.Bacc("TRN2", target_bir_lowering=True, debug=False)
    else:
        nc = bass.Bass("TRN2", target_bir_lowering=False, debug=False)

    # ---- DRAM I/O ----
    encT_d = nc.dram_tensor("encT", [128, t_enc * 128], bf, kind="ExternalInput").ap()
    h0_d = nc.dram_tensor("h0T", [128, 4 * BC], bf, kind="ExternalInput").ap()
    m1_d = nc.dram_tensor("m1T", [128, 16 * 128], bf, kind="ExternalInput").ap()
    wih_d = nc.dram_tensor("wihT", [128, 48 * 128], bf, kind="ExternalInput").ap()
    whh_d = nc.dram_tensor("whhT", [128, 48 * 128], bf, kind="ExternalInput").ap()
    emb_d = nc.dram_tensor("embT", [128, 8 * 128], bf, kind="ExternalInput").ap()
    wyd_d = nc.dram_tensor("wydT", [128, 4 * O], bf, kind="ExternalInput").ap()
    regw_d = nc.dram_tensor("regwT", [128, 2 * O], bf, kind="ExternalInput").ap()
    # bias tiles [128,128]: tile j, partition-row k<4 holds the k-th quarter
    # of that bias vector.  Order: E(c_e) E0(emb_b) R Z HN IN
    bias_d = nc.dram_tensor("biasT", [128, 6 * 128], bf, kind="ExternalInput").ap()
    # indicator rhs: cols 0:256 ind[k, j] = 1 iff j//64 == k (k<4);
    # cols 256:384 ind[k, j] = 1 iff k == 2 + j//64 (for the e_hi bank)
    ind_d = nc.dram_tensor("indT", [128, 384], bf, kind="ExternalInput").ap()
    # y-bias rhs tiles [128, 2*O]: row 0 of block 0 = reg_b, block 1 = c_yd
    ybias_d = nc.dram_tensor("ybT", [128, 2 * O], bf, kind="ExternalInput").ap()
    # token-major output: one token = [BC, O] f32 contiguous 16KB, so each
    # y DMA is a single dense block (the [BC, t, O] layout needed 64
    # strided 256B rows per token and backed up the DMA queue ~10us by
    # kernel end).  The host transposes back to [BC, t, O].
    y_d = nc.dram_tensor("y", [t_enc + nsteps, BC, O], f32, kind="ExternalOutput").ap()

    with tile.TileContext(nc) as tc:
        import contextlib
        with contextlib.ExitStack() as ctx:
            consts = ctx.enter_context(tc.tile_pool(name="consts", bufs=1))
            temps = ctx.enter_context(tc.tile_pool(name="temps", bufs=2))
            ytmp = ctx.enter_context(tc.tile_pool(name="ytmp", bufs=3))
            psum = ctx.enter_context(tc.tile_pool(name="psum", bufs=1, space="PSUM"))

            # ---- ACT table warmup (pin the table load to dependency-light
            # dummy ops) ----
            wt = consts.tile([128, 10], f32, tag="wtbl", name="wtbl")
            nc.vector.memset(wt[:, 0:5], 0.0)
            nc.scalar.activation(wt[:, 5:6], wt[:, 0:1], AF.Relu)
            nc.scalar.activation(wt[:, 6:7], wt[:, 1:2], AF.Sigmoid)
            nc.scalar.activation(wt[:, 7:8], wt[:, 2:3], AF.Tanh)
            nc.scalar.activation(wt[:, 8:9], wt[:, 3:4], AF.Identity)

            # ---- load constants into SBUF ----
            m1_sb = consts.tile([128, 16 * 128], bf, tag="m1")
            wih_sb = consts.tile([128, 48 * 128], bf, tag="wih")
            whh_sb = consts.tile([128, 48 * 128], bf, tag="whh")
            emb_sb = consts.tile([128, 8 * 128], bf, tag="emb")
            wyd_sb = consts.tile([128, 4 * O], bf, tag="wyd")
            regw_sb = consts.tile([128, 2 * O], bf, tag="regw")
            bias_sb = consts.tile([128, 6 * 128], bf, tag="biasT")
            ind_sb = consts.tile([128, 384], bf, tag="indT")
            ybias_sb = consts.tile([128, 2 * O], bf, tag="ybT")
            encT_sb = consts.tile([128, t_enc * 128], bf, tag="encT")

            # ---- persistent state: h + a constant ones chunk at [4BC:5BC]
            # (lhsT for the y-bias matmul) ----
            h_sb = consts.tile([128, 5 * BC], bf, tag="h", name="h")
            nc.vector.memset(h_sb[:, 4 * BC:5 * BC], 1.0)
            h_ones = h_sb[:, 4 * BC:5 * BC]

            # DMA order = step-0 readiness order: bias/ind gate the bank
            # arming, emb+x0 gate A(0), h0+whh gate Br, wih gates Cr.  m1 is
            # not needed until step 1, the encoder body streams in behind.
            lastblk = slice((t_enc - 1) * 128, t_enc * 128)
            nc.sync.dma_start(out=bias_sb, in_=bias_d)
            nc.sync.dma_start(out=ind_sb, in_=ind_d)
            nc.sync.dma_start(out=encT_sb[:, lastblk], in_=encT_d[:, lastblk])
            nc.sync.dma_start(out=emb_sb, in_=emb_d)
            nc.sync.dma_start(out=h_sb[:, 0:4 * BC], in_=h0_d)
            nc.sync.dma_start(out=whh_sb, in_=whh_d)
            nc.sync.dma_start(out=wih_sb, in_=wih_d)
            nc.sync.dma_start(out=regw_sb, in_=regw_d)
            nc.sync.dma_start(out=ybias_sb, in_=ybias_d)
            nc.sync.dma_start(out=m1_sb, in_=m1_d)
            nc.sync.dma_start(out=wyd_sb, in_=wyd_d)
            nsplit = 4
            per = (t_enc - 1) // nsplit + 1
            for i in range(nsplit):
                lo, hi = i * per, min((i + 1) * per, t_enc - 1)
                if lo >= hi:
                    continue
                nc.sync.dma_start(out=encT_sb[:, lo * 128:hi * 128],
                                  in_=encT_d[:, lo * 128:hi * 128])

            # ---- persistent PSUM regions, one bank each.  The e pre-act is
            # split across TWO banks (quarters 0,1 / 2,3) so relu_lo can
            # start as soon as the first half of A closes, shortening the
            # A -> relu -> Cr chain prefix.  y gets a single bank to pay
            # for it (its copy/bias WAR coupling has ample slack). ----
            el_ps = psum.tile([128, 512], f32, tag="elps")     # e quarters 0,1
            eh_ps = psum.tile([128, 512], f32, tag="ehps")     # e quarters 2,3
            r_ps = psum.tile([128, 512], f32, tag="rps")       # a_r
            hn_ps = psum.tile([128, 512], f32, tag="hnps")     # hn
            in_ps = psum.tile([128, 512], f32, tag="inps")     # inn
            z_pp = [psum.tile([128, 512], f32, tag=f"zps{s}", name=f"zps{s}")
                    for s in range(2)]
            y_ps = psum.tile([BC, 512], f32, tag="yps")

            def wtile(sb, mi, k, n_k):
                j = (mi * n_k + k) * 128
                return sb[:, j:j + 128]

            # bias tile indices in bias_sb
            BE, BE0, BR, BZ, BHN, BIN = 0, 1, 2, 3, 4, 5

            def btile(idx):
                return bias_sb[:, idx * 128:(idx + 1) * 128]

            cs = lambda m: slice(m * BC, (m + 1) * BC)

            def emit_biases(t):
                """Arm all PSUM banks for step t with their bias values.
                Emitted in step t-1's tail: none has an outstanding
                dependency there, so they fill the PE's h'-wait window."""
                eb = BE0 if t == 0 else BE
                nc.tensor.matmul(el_ps[:, 0:128], btile(eb), ind_sb[:, 0:128],
                                 start=True, stop=False)
                nc.tensor.matmul(eh_ps[:, 0:128], btile(eb),
                                 ind_sb[:, 256:384], start=True, stop=False)
                nc.tensor.matmul(r_ps[:, 0:256], btile(BR), ind_sb[:, 0:256],
                                 start=True, stop=False)
                nc.tensor.matmul(hn_ps[:, 0:256], btile(BHN), ind_sb[:, 0:256],
                                 start=True, stop=False)
                nc.tensor.matmul(in_ps[:, 0:256], btile(BIN), ind_sb[:, 0:256],
                                 start=True, stop=False)
                nc.tensor.matmul(z_pp[t % 2][:, 0:256], btile(BZ),
                                 ind_sb[:, 0:256], start=True, stop=False)

            def gate_mms(dst, w_sb, m_base, src, n_k, stop_last,
                         k_major=False):
                """One gate region: 4 output quarters x n_k contraction
                chunks, continuing the bank group armed by its bias mm.
                k_major orders the k=0,1 matmuls first so a relu_lo-gated
                group can start before relu_hi lands."""
                order = [(m, k) for k in range(n_k) for m in range(4)] \
                    if k_major else [(m, k) for m in range(4)
                                     for k in range(n_k)]
                for i, (m, k) in enumerate(order):
                    nc.tensor.matmul(dst[:, cs(m)],
                                     wtile(w_sb, m_base + m, k, n_k),
                                     src[:, k * BC:(k + 1) * BC],
                                     start=False,
                                     stop=(stop_last and i == len(order) - 1))

            def emit_e_mms(t):
                """A-group, split across the two e banks: quarters 0,1 into
                el_ps first (closing it early for relu_lo), then 2,3."""
                w_sb, n_k = (emb_sb, 2) if t == 0 else (m1_sb, 4)
                src = encT_sb[:, lastblk] if t == 0 else h_sb
                for half, dst in ((0, el_ps), (1, eh_ps)):
                    for m in range(2):
                        for k in range(n_k):
                            nc.tensor.matmul(
                                dst[:, cs(m)],
                                wtile(w_sb, half * 2 + m, k, n_k),
                                src[:, k * BC:(k + 1) * BC],
                                start=False,
                                stop=(m == 1 and k == n_k - 1))

            # ---- PE pre-warm: ~40 dependency-free dummy matmuls keep the
            # PE busy through the weight-DMA phase so HAM un-throttles to
            # 2.4 GHz (~4us of sustained activity) BEFORE step 0's chain,
            # instead of ~24us in (the first 4 steps otherwise run at
            # 1.2 GHz).  They write the y bank, which the y-bias matmul
            # below overwrites afterwards in FIFO order. ----
            dumm = consts.tile([128, 512], bf, tag="dumm", name="dumm")
            nc.vector.memset(dumm, 0.0)
            for _ in range(40):
                nc.tensor.matmul(y_ps[:, 0:512], dumm[:, 0:64],
                                 dumm[:, 0:512], start=True, stop=True)

            emit_biases(0)
            nc.tensor.matmul(y_ps[:, 0:128], h_ones, ybias_sb,
                             start=True, stop=False)

            for t in range(nsteps):
                e_t = temps.tile([128, 256], bf, tag="e")
                r_t = temps.tile([128, 256], bf, tag="r")
                z_t = temps.tile([128, 256], bf, tag="z")
                n_t = temps.tile([128, 256], bf, tag="n")
                t3 = temps.tile([128, 256], bf, tag="t3")
                t4 = temps.tile([128, 256], f32, tag="t4")
                d_t = temps.tile([128, 256], bf, tag="d")
                t6 = temps.tile([128, 256], bf, tag="t6")

                z_ps = z_pp[t % 2]

                # A: e pre-activation (h-gated; step 0 uses emb @ x0),
                # half el closes first
                emit_e_mms(t)
                # Br: whh r-half (h-gated, group stays open)
                gate_mms(r_ps, whh_sb, 0, h_sb, 4, False)
                # relu on DVE, by e-bank half (unloads ACT; lets Cr's k=0,1
                # matmuls start before the hi half lands)
                nc.vector.tensor_scalar_max(e_t[:, 0:128], el_ps[:, 0:128], 0.0)
                nc.vector.tensor_scalar_max(e_t[:, 128:256], eh_ps[:, 0:128], 0.0)
                # Cr: wih r-half closes the r bank (relu-gated, k-major)
                gate_mms(r_ps, wih_sb, 0, e_t, 4, True, k_major=True)
                # sigma_r (ACT)
                nc.scalar.activation(r_t, r_ps[:, 0:256], AF.Sigmoid)
                # D: hn region (h-gated)
                gate_mms(hn_ps, whh_sb, 8, h_sb, 4, True)
                # E: inn region (relu-gated)
                gate_mms(in_ps, wih_sb, 8, e_t, 4, True)
                # t3 = hn * r ; t4 = t3 + inn  (DVE)
                nc.vector.tensor_tensor(t3, hn_ps[:, 0:256], r_t, OP.mult)
                nc.vector.tensor_tensor(t4, t3, in_ps[:, 0:256], OP.add)
                # Bz / Cz: z gate last (shortest post-closure chain depth)
                gate_mms(z_ps, whh_sb, 4, h_sb, 4, False)
                gate_mms(z_ps, wih_sb, 4, e_t, 4, True)
                # n = tanh(t4); sigma_z after it in the ACT queue
                nc.scalar.activation(n_t, t4, AF.Tanh)
                nc.scalar.activation(z_t, z_ps[:, 0:256], AF.Sigmoid)
                # y matmuls (read h(t) = current h_sb, so they must be
                # emitted before the DVE tail overwrites it; they execute
                # during the sigma_z window)
                if t > 0:
                    for k in range(4):
                        nc.tensor.matmul(y_ps[:, 64:128],
                                         h_sb[:, k * BC:(k + 1) * BC],
                                         wyd_sb[:, k * O:(k + 1) * O],
                                         start=False,
                                         stop=(k == 3 and t >= t_enc))
                if t < t_enc:
                    for k in range(2):
                        nc.tensor.matmul(
                            y_ps[:, 0:64],
                            encT_sb[:, t * 128 + k * BC: t * 128 + (k + 1) * BC],
                            regw_sb[:, k * O:(k + 1) * O],
                            start=False, stop=(k == 1))
                # DVE tail: d = h - n; t6 = z*d; h' = n + t6 (in-place)
                nc.vector.tensor_tensor(d_t, h_sb[:, 0:256], n_t, OP.subtract)
                nc.vector.tensor_tensor(t6, z_t, d_t, OP.mult)
                nc.vector.tensor_tensor(h_sb[:, 0:256], n_t, t6, OP.add)
                # next step's bias matmuls: dependency-free here, they fill
                # the PE's wait-for-h' window
                if t + 1 < nsteps:
                    emit_biases(t + 1)

                # ---------------- y staging + DMA ----------------
                if t > 0 and t < t_enc:
                    y_all = ytmp.tile([BC, 128], f32, tag="yall")
                    nc.scalar.activation(y_all, y_ps[:, 0:128], AF.Identity)
                    nc.sync.dma_start(out=y_d[t, :, :], in_=y_all[:, 0:64])
                    nc.sync.dma_start(out=y_d[t_enc + t - 1, :, :],
                                      in_=y_all[:, 64:128])
                elif t < t_enc:  # t == 0: encoder token only
                    y_enc = ytmp.tile([BC, O], f32, tag="yenc")
                    nc.scalar.activation(y_enc, y_ps[:, 0:64], AF.Identity)
                    nc.sync.dma_start(out=y_d[t, :, :], in_=y_enc)
                else:            # t >= t_enc: decoder token only
                    y_dec = ytmp.tile([BC, O], f32, tag="ydec")
                    nc.scalar.activation(y_dec, y_ps[:, 64:128], AF.Identity)
                    nc.sync.dma_start(out=y_d[t_enc + t - 1, :, :], in_=y_dec)
                # y-bias for step t+1 (single y bank: must be emitted AFTER
                # this step's ycopy so its WAR pins it behind the copy --
                # the scheduler would otherwise hoist the start=True matmul
                # over this step's y accumulation and wipe it)
                if t + 1 < nsteps:
                    nc.tensor.matmul(y_ps[:, 0:128], h_ones, ybias_sb,
                                     start=True, stop=False)

            # final decoder token from h(nsteps)
            nc.tensor.matmul(y_ps[:, 64:128], h_ones, ybias_sb[:, O:2 * O],
                             start=True, stop=False)
            for k in range(4):
                nc.tensor.matmul(y_ps[:, 64:128],
                                 h_sb[:, k * BC:(k + 1) * BC],
                                 wyd_sb[:, k * O:(k + 1) * O],
                                 start=False, stop=(k == 3))
            y_fin = ytmp.tile([BC, O], f32, tag="ydec")
            nc.scalar.activation(y_fin, y_ps[:, 64:128], AF.Identity)
            nc.sync.dma_start(out=y_d[t_enc + nsteps - 1, :, :], in_=y_fin)

            # leftover encoder tokens if nsteps < t_enc (smoke tests only)
            for t in range(nsteps, t_enc):
                nc.tensor.matmul(y_ps[:, 0:64], h_ones, ybias_sb[:, 0:O],
                                 start=True, stop=False)
                for k in range(2):
                    nc.tensor.matmul(
                        y_ps[:, 0:64],
                        encT_sb[:, t * 128 + k * BC: t * 128 + (k + 1) * BC],
                        regw_sb[:, k * O:(k + 1) * O],
                        start=False, stop=(k == 1))
                y_enc = ytmp.tile([BC, O], f32, tag="yenc")
                nc.scalar.activation(y_enc, y_ps[:, 0:64], AF.Identity)
                nc.sync.dma_start(out=y_d[t, :, :], in_=y_enc)

    if lowering:
        nc.finalize()
    return nc


def prep_inputs(encoder_outputs, encoder_hidden, emb_W, emb_b, w_ih, w_hh,
                b_ih, b_hh, out_W, out_b, reg_W, reg_b, nsteps=PRED_LEN,
                t_enc=T_ENC):
    """Host-side packing. Returns per-core input dicts."""
    f32 = np.float32
    f64 = np.float64
    emb_W, emb_b, w_ih, w_hh, b_ih, b_hh, out_W, out_b, reg_W, reg_b = (
        np.asarray(a, f32) for a in
        (emb_W, emb_b, w_ih, w_hh, b_ih, b_hh, out_W, out_b, reg_W, reg_b))

    # fused weights (x = h@out_W.T + out_b is linear in h)
    M1 = (out_W.T.astype(f64) @ emb_W.T.astype(f64)).astype(f32)     # [H, H]
    c_e = (emb_b + out_b @ emb_W.T).astype(f32)                      # [H]
    Wyd = (out_W.T.astype(f64) @ reg_W.T.astype(f64)).astype(f32)    # [H, O]
    c_yd = (out_b @ reg_W.T + reg_b).astype(f32)                     # [O]

    # bias tiles [128, 6*128]: tile j, partition-row k<4 = quarter k of the
    # bias vector (order E, E0, R, Z, HN, IN)
    bias_rows = np.stack([
        c_e.reshape(4, 128),
        emb_b.reshape(4, 128),
        (b_ih[:H] + b_hh[:H]).reshape(4, 128),
        (b_ih[H:2 * H] + b_hh[H:2 * H]).reshape(4, 128),
        b_hh[2 * H:].reshape(4, 128),
        b_ih[2 * H:].reshape(4, 128),
    ])                                                   # [6, 4, 128]
    bias_pack = np.zeros((128, 6 * 128), f32)
    for j in range(6):
        for k in range(4):
            bias_pack[k, j * 128:(j + 1) * 128] = bias_rows[j, k]

    # indicator rhs: cols 0:256 ind[k, j] = 1 iff j//64 == k; cols 256:384
    # select quarters 2,3 for the e_hi bank
    ind_pack = np.zeros((128, 384), f32)
    for k in range(4):
        ind_pack[k, k * 64:(k + 1) * 64] = 1.0
    for k in range(2):
        ind_pack[2 + k, 256 + k * 64:256 + (k + 1) * 64] = 1.0

    yb_pack = np.zeros((128, 2 * O), f32)
    yb_pack[0, 0:O] = reg_b
    yb_pack[0, O:2 * O] = c_yd

    shared = {
        "m1T": _pack_tiles(M1, 4, 4),
        "wihT": _pack_tiles(w_ih.T, 4, 12),
        "whhT": _pack_tiles(w_hh.T, 4, 12),
        "embT": _pack_tiles(emb_W.T, 2, 4),
        "wydT": np.ascontiguousarray(
            Wyd.reshape(4, 128, O).transpose(1, 0, 2).reshape(128, 4 * O)
            .astype(bf16)),
        "regwT": np.ascontiguousarray(
            reg_W.T.reshape(2, 128, O).transpose(1, 0, 2).reshape(128, 2 * O)
            .astype(bf16)),
        "biasT": np.ascontiguousarray(bias_pack.astype(bf16)),
        "indT": np.ascontiguousarray(ind_pack.astype(bf16)),
        "ybT": np.ascontiguousarray(yb_pack.astype(bf16)),
    }

    enc = np.asarray(encoder_outputs, f32)[:, :t_enc, :]
    h0 = np.asarray(encoder_hidden, f32)[0]
    in_maps = []
    for i in range(NCORES):
        sl = slice(i * BC, (i + 1) * BC)
        enc_i = enc[sl].astype(bf16)              # [BC, t_enc, E]
        encT = (enc_i.reshape(BC, t_enc, 2, 128).transpose(3, 1, 2, 0)
                .reshape(128, t_enc * 128))
        m = dict(shared)
        m["encT"] = np.ascontiguousarray(encT)
        m["h0T"] = _feat_major(h0[sl], 4).astype(bf16)
        in_maps.append(m)
    return in_maps


def kernel(encoder_outputs, encoder_hidden, emb_W, emb_b, w_ih, w_hh,
           b_ih, b_hh, out_W, out_b, reg_W, reg_b):
    from concourse.bass_utils import run_bass_kernel_spmd

    nc = build_program()
    in_maps = prep_inputs(encoder_outputs, encoder_hidden, emb_W, emb_b,
                          w_ih, w_hh, b_ih, b_hh, out_W, out_b, reg_W, reg_b)
    res = run_bass_kernel_spmd(nc, in_maps, core_ids=list(range(NCORES)))
    out = np.empty((B, T_ALL, O), np.float32)
    for i in range(NCORES):
        out[i * BC:(i + 1) * BC] = res.results[i]["y"].transpose(1, 0, 2)
    return out
